# revision 40
# baseline (speedup 1.0000x reference)
"""MultiHeadAttention Trainium2 kernel, 8-way sharded (batch x head-group).

Sharding: core = 4*b + g  (b in {0,1} batch, g in {0..3} head-group of 4 heads).
Data parallel on batch; tensor parallel on heads for the Q/K/V projections
with a row-parallel Wo output projection.  Each core computes a full-shape
partial output for its batch (bias folded in on one core per group); the
host-side unshard step sums the 4 head-group partials per batch.

Device-side structure (all bf16 matmul operands, f32 PSUM accumulate):
  - x arrives pre-transposed and pre-cast on the host (xT [1024, 2048] bf16
    per batch): zero on-device transposes or input casts.  Weights arrive
    bf16 in SBUF-native layouts (one contiguous DMA line per partition).
  - Software pipeline over 512-row chunks, attention ascending 0..3; the
    K/V/Q projections of later chunks and the output projections of earlier
    chunks are interleaved into the ACT-bound attention kt-steps as small
    "filler" units, which keeps the PE dense (HAM stays un-throttled) and
    the scores->exp->PV chain saturated.  Emission-order gates (`ensure_kv`)
    guarantee a chunk's K/V projections are emitted before the kt steps
    that read them.
  - exp batched over both heads of a pair: scores for 2 heads land in one
    [128,2,512] PSUM tile (2 banks), one ACTIVATE computes both.
  - Softmax denominator folded into PV via an augmented ones column in V
    (row 64 of the PV accumulator); normalize copies the 65-row result out
    of PSUM in one DVE op (frees the bank fast), then reciprocal + gpsimd
    partition-broadcast + two DVE mults.
"""
import sys

for _p in ("/opt/trn_rl_repo",):
    if _p not in sys.path:
        sys.path.insert(0, _p)

from collections import deque

import numpy as np
import ml_dtypes

import concourse.bass as bass
import concourse.tile as tile
from concourse import bacc, mybir
from concourse.bass_utils import run_bass_kernel_spmd


def _install_ntff_hook_shim():
    """The agent container's antenv lacks axon_hooks; recreate it so
    run_bass_kernel_spmd(trace=True) can profile via the axon .so."""
    import types, contextlib, ctypes, os

    if "antenv.axon_hooks" in sys.modules:
        return
    mod = types.ModuleType("antenv.axon_hooks")
    _store = {"hook": None}
    mod.set_axon_ntff_profile_hook = lambda h: _store.__setitem__("hook", h)
    mod.get_axon_ntff_profile_hook = lambda: _store["hook"]
    sys.modules["antenv.axon_hooks"] = mod

    so_path = "/opt/axon/libaxon_pjrt.so"
    if not os.path.exists(so_path):
        return
    try:
        lib = ctypes.CDLL(so_path)
        if not hasattr(lib, "axon_start_nrt_profile"):
            return
        lib.axon_start_nrt_profile.argtypes = [
            ctypes.POINTER(ctypes.c_int64), ctypes.c_size_t]
        lib.axon_start_nrt_profile.restype = ctypes.c_int64
        lib.axon_stop_nrt_profile.argtypes = [ctypes.c_char_p]
        lib.axon_stop_nrt_profile.restype = ctypes.c_int64

        @contextlib.contextmanager
        def _hook(output_dir, device_ids):
            import jax
            jax.devices()
            if device_ids:
                ids = (ctypes.c_int64 * len(device_ids))(*device_ids)
                rc = lib.axon_start_nrt_profile(ids, len(device_ids))
            else:
                rc = lib.axon_start_nrt_profile(None, 0)
            if rc != 0:
                raise RuntimeError(f"axon_start_nrt_profile rc={rc}")
            try:
                yield
            finally:
                n = lib.axon_stop_nrt_profile(str(output_dir).encode())
                print(f"ntff profile: {n} file(s) written to {output_dir}")

        mod.set_axon_ntff_profile_hook(_hook)
    except Exception:
        pass


_install_ntff_hook_shim()

F32 = mybir.dt.float32
BF16 = mybir.dt.bfloat16
AF = mybir.ActivationFunctionType
ALU = mybir.AluOpType

B, S, D_EMB = 2, 2048, 1024
H, DH = 16, 64
HG = 4              # heads per core
DM_L = HG * DH      # 256 local mid dim
D_OUT = 1024
NCORES = 8
ET = D_EMB // 128   # 8 emb tiles
QC = 4              # q chunks of 512
SCALE = 1.0 / 8.0   # 1/sqrt(DH)

# augmented V layout: per head slice [v(64), one] -> PV output rows 0..63 = O,
# row 64 = softmax denominator (the ones column sums P over keys).
HOFF = [0, 65, 130, 195]
WV_AUG = 272        # 260 used + pad
WV_USED = 260


def _build():
    nc = bacc.Bacc(None, target_bir_lowering=False, num_devices=NCORES)

    xqT = nc.declare_dram_parameter("xqT", [D_EMB, S], BF16, isOutput=False)
    xkT = nc.declare_dram_parameter("xkT", [D_EMB, S], BF16, isOutput=False)
    xvT = nc.declare_dram_parameter("xvT", [D_EMB, S], BF16, isOutput=False)
    wq = nc.declare_dram_parameter("wq", [128, ET * DM_L], BF16, isOutput=False)
    wk = nc.declare_dram_parameter("wk", [128, ET * DM_L], BF16, isOutput=False)
    wv = nc.declare_dram_parameter("wv", [128, ET * WV_AUG], BF16, isOutput=False)
    bq = nc.declare_dram_parameter("bq", [DM_L], F32, isOutput=False)
    bk = nc.declare_dram_parameter("bk", [DM_L], F32, isOutput=False)
    bv = nc.declare_dram_parameter("bv", [WV_AUG], F32, isOutput=False)
    wo = nc.declare_dram_parameter("wo", [128, 2 * D_OUT], BF16, isOutput=False)
    bo = nc.declare_dram_parameter("bo", [D_OUT], F32, isOutput=False)
    mtri = nc.declare_dram_parameter("mtri", [128, 128], BF16, isOutput=False)
    out = nc.declare_dram_parameter("out", [S, D_OUT], BF16, isOutput=True)

    with tile.TileContext(nc) as tc:
        _emit(nc, tc, xqT.ap(), xkT.ap(), xvT.ap(), wq.ap(), wk.ap(), wv.ap(),
              bq.ap(), bk.ap(), bv.ap(), wo.ap(), bo.ap(), mtri.ap(), out.ap())
    nc.compile()
    return nc


def _emit(nc, tc, xqT, xkT, xvT, wq, wk, wv, bq, bk, bv, wo, bo, mtri, out):
    from contextlib import ExitStack

    ctx = ExitStack()
    consts = ctx.enter_context(tc.tile_pool(name="consts", bufs=1))
    wpool = ctx.enter_context(tc.tile_pool(name="wpool", bufs=1))
    persist = ctx.enter_context(tc.tile_pool(name="persist", bufs=1))
    xload = ctx.enter_context(tc.tile_pool(name="xload", bufs=6))
    ptp = ctx.enter_context(tc.tile_pool(name="ptp", bufs=3))
    ocpp = ctx.enter_context(tc.tile_pool(name="ocpp", bufs=2))
    smallp = ctx.enter_context(tc.tile_pool(name="smallp", bufs=4))
    outp = ctx.enter_context(tc.tile_pool(name="outp", bufs=4))
    ps_sc = ctx.enter_context(tc.tile_pool(name="ps_sc", bufs=2, space="PSUM"))
    ps_po = ctx.enter_context(tc.tile_pool(name="ps_po", bufs=1, space="PSUM"))
    ps_pp = ctx.enter_context(tc.tile_pool(name="ps_pp", bufs=2, space="PSUM"))

    # ---- constants ----
    mtri_sb = consts.tile([128, 128], BF16)
    nc.scalar.dma_start(mtri_sb[:], mtri[:])

    # PE warm-up: back-to-back matmuls so HAM unthrottles while DMAs land.
    warm_ps = ps_pp.tile([128, 512], F32, tag="pp", name="warm")
    for _ in range(28):
        nc.tensor.matmul(
            warm_ps[:, 0:128], lhsT=mtri_sb[:], rhs=mtri_sb[:],
            start=True, stop=True,
        )

    # preload the exp table early (first ACTIVATE triggers the table DMA)
    dummy_f32 = consts.tile([1, 16], F32)
    nc.vector.memset(dummy_f32[:], 0.0)
    dummy_o = consts.tile([1, 16], F32)
    nc.scalar.activation(out=dummy_o[:], in_=dummy_f32[:], func=AF.Exp, scale=1.0)

    # biases: bq/bk as per-partition columns [128, 2] (c2-major) folded into
    # the qT/kT evacuations; bv broadcast to all partitions and folded into
    # the V evacuation.
    bk_sb = consts.tile([128, 2], F32, name="bk")
    nc.scalar.dma_start(bk_sb[:], bk.rearrange("(c p) -> p c", p=128))
    bq_sb = consts.tile([128, 2], F32, name="bq")
    nc.scalar.dma_start(bq_sb[:], bq.rearrange("(c p) -> p c", p=128))
    bv_row = consts.tile([1, WV_AUG], F32, name="bv_row")
    nc.scalar.dma_start(bv_row[:], bv[None, :])
    bv_bc = consts.tile([128, WV_AUG], F32, name="bv_bc")
    nc.gpsimd.partition_broadcast(bv_bc[:], bv_row[:])

    # bo broadcast to 128 partitions (f32); folded into the output-projection
    # partials (only core g==0 of each group gets a nonzero bo).
    bo_bc = consts.tile([128, D_OUT], F32)
    bo_bcast_ap = bass.AP(tensor=bo.tensor, offset=bo.offset, ap=[[0, 128], [1, D_OUT]])
    nc.gpsimd.dma_start(out=bo_bc[:], in_=bo_bcast_ap)

    # ---- weights (bf16, SBUF-native layout, scalar DMA queue) ----
    wk_sb = wpool.tile([128, ET, DM_L], BF16, name="wk")
    nc.scalar.dma_start(wk_sb[:], wk.rearrange("p (t d) -> p t d", t=ET))
    wv_sb = wpool.tile([128, ET, WV_AUG], BF16, name="wv")
    nc.scalar.dma_start(wv_sb[:], wv.rearrange("p (t d) -> p t d", t=ET))
    wq_sb = wpool.tile([128, ET, DM_L], BF16, name="wq")
    nc.scalar.dma_start(wq_sb[:], wq.rearrange("p (t d) -> p t d", t=ET))
    wo_sb = wpool.tile([128, 2, D_OUT], BF16, name="wo")
    nc.scalar.dma_start(wo_sb[:], wo.rearrange("p (t d) -> p t d", t=2))

    # ---- persistent attention operands ----
    qT = [persist.tile([128, S], BF16, name=f"qT{i}") for i in range(2)]
    kT = [persist.tile([128, S], BF16, name=f"kT{i}") for i in range(2)]
    v_sb = persist.tile([128, 4 * QC, WV_AUG], BF16)
    s1T = persist.tile([128, 2, S], BF16, name="s1T")

    # ---- xT chunk loads: one DMA per (input, chunk), ascending chunks ----
    xsrc = {"q": xqT, "k": xkT, "v": xvT}
    xtiles = {}

    def load_x(key, qc):
        t = xload.tile([128, ET, 512], BF16, tag="xT", name=f"xT_{key}{qc}")
        src = xsrc[key].rearrange("(t p) s -> p t s", p=128)
        # two DMAs per tile so the projection can start on the first half
        nc.sync.dma_start(t[:, 0:4, :], src[:, 0:4, qc * 512:(qc + 1) * 512])
        nc.sync.dma_start(t[:, 4:8, :], src[:, 4:8, qc * 512:(qc + 1) * 512])
        xtiles[(key, qc)] = t

    for qc in range(QC):
        load_x("k", qc)
        load_x("q", qc)
        load_x("v", qc)

    # ---- filler machinery: small PE units interleaved into attention.
    # Emission order IS dependency order for the Tile framework, so a unit
    # that writes data consumed by an attention step must be emitted before
    # that step: `ensure_kv` force-pops K/V projection units up to the chunk
    # a kt step is about to read. ----
    fillers = deque()
    pending = {}

    def fadd(cost, fn, gate=None):
        fillers.append((cost, fn, gate))
        if gate is not None:
            pending[gate] = pending.get(gate, 0) + 1

    def pop_one():
        cost, fn, gate = fillers.popleft()
        fn()
        if gate is not None:
            pending[gate] -= 1
        return cost

    def pop_fillers(budget):
        while fillers and budget > 0.0:
            budget -= pop_one()

    def ensure_g(key):
        while fillers and pending.get(key, 0):
            pop_one()

    def drain_fillers():
        while fillers:
            pop_one()

    def add_projT_units(qc, c2, w_sb, b_sb, dst):
        """qT/kT projection for mid-half c2 of chunk qc -> dst[c2] columns."""
        st = {}
        kind = "Q" if dst is qT else "K"
        gate = (kind, qc, c2)
        xt = xtiles[(kind.lower(), qc)]

        def u_start(st=st, xt=xt, c2=c2, w_sb=w_sb):
            pp = ps_pp.tile([128, 512], F32, tag="pp")
            st["pp"] = pp
            nc.tensor.matmul(
                pp[:, 0:512],
                lhsT=w_sb[:, 0, c2 * 128:(c2 + 1) * 128],
                rhs=xt[:, 0, :],
                start=True, stop=False,
            )
        fadd(0.25, u_start, gate)

        for e0 in (1, 3, 5):
            def u_mid(st=st, xt=xt, c2=c2, w_sb=w_sb, e0=e0):
                for ei in (e0, e0 + 1):
                    nc.tensor.matmul(
                        st["pp"][:, 0:512],
                        lhsT=w_sb[:, ei, c2 * 128:(c2 + 1) * 128],
                        rhs=xt[:, ei, :],
                        start=False, stop=False,
                    )
            fadd(0.45, u_mid, gate)

        def u_end(st=st, xt=xt, c2=c2, w_sb=w_sb, dst=dst, qc=qc, b_sb=b_sb):
            nc.tensor.matmul(
                st["pp"][:, 0:512],
                lhsT=w_sb[:, ET - 1, c2 * 128:(c2 + 1) * 128],
                rhs=xt[:, ET - 1, :],
                start=False, stop=True,
            )
            nc.vector.tensor_scalar(
                out=dst[c2][:, qc * 512:(qc + 1) * 512],
                in0=st["pp"][:, 0:512],
                scalar1=b_sb[:, c2:c2 + 1],
                scalar2=None,
                op0=ALU.add,
            )
        fadd(0.45, u_end, gate)

    def add_projV_units(qc, r):
        """V projection for 128-row block r of chunk qc -> v_sb natural."""
        st = {}
        si = 4 * qc + r
        xt = xtiles[("v", qc)]

        def u_start(st=st, xt=xt, r=r):
            pp = ps_pp.tile([128, 512], F32, tag="pp")
            st["pp"] = pp
            nc.tensor.matmul(
                pp[:, 0:WV_USED],
                lhsT=xt[:, 0, r * 128:(r + 1) * 128],
                rhs=wv_sb[:, 0, 0:WV_USED],
                start=True, stop=False,
            )
        fadd(0.15, u_start, ("V", qc, r))

        for e0 in (1, 3, 5):
            def u_mid(st=st, xt=xt, r=r, e0=e0):
                for ei in (e0, e0 + 1):
                    nc.tensor.matmul(
                        st["pp"][:, 0:WV_USED],
                        lhsT=xt[:, ei, r * 128:(r + 1) * 128],
                        rhs=wv_sb[:, ei, 0:WV_USED],
                        start=False, stop=False,
                    )
            fadd(0.3, u_mid, ("V", qc, r))

        def u_end(st=st, xt=xt, r=r, si=si):
            nc.tensor.matmul(
                st["pp"][:, 0:WV_USED],
                lhsT=xt[:, ET - 1, r * 128:(r + 1) * 128],
                rhs=wv_sb[:, ET - 1, 0:WV_USED],
                start=False, stop=True,
            )
            nc.vector.tensor_tensor(
                out=v_sb[:, si, 0:WV_USED],
                in0=st["pp"][:, 0:WV_USED],
                in1=bv_bc[:, 0:WV_USED],
                op=ALU.add,
            )
        fadd(0.3, u_end, ("V", qc, r))

    def add_proj_chunk(qc):
        add_projT_units(qc, 0, wk_sb, bk_sb, kT)
        add_projT_units(qc, 0, wq_sb, bq_sb, qT)
        for r in range(4):
            add_projV_units(qc, r)
        add_projT_units(qc, 1, wk_sb, bk_sb, kT)
        add_projT_units(qc, 1, wq_sb, bq_sb, qT)

    def add_outproj_chunk(qc, use_sc=False):
        """Output projection for 512-row block qc; partial rows go straight
        to the output parameter (host sums the 4 head-group partials)."""
        for r in range(4):
            si = 4 * qc + r
            st = {}

            def u_alloc(st=st):
                st["ob"] = outp.tile([128, D_OUT], BF16, tag="ob", name="ob")
            fadd(0.0, u_alloc)

            for half in range(2):
                def u_half(st=st, si=si, half=half, r=r):
                    if use_sc and (r + half) % 2 == 0:
                        ppt = ps_sc.tile([128, 2, 512], F32, tag="st", name="pp2")
                        pp = ppt[:, 0, 0:512]
                    else:
                        ppt = ps_pp.tile([128, 512], F32, tag="pp")
                        pp = ppt[:, 0:512]
                    for c2 in range(2):
                        nc.tensor.matmul(
                            pp,
                            lhsT=s1T[:, c2, si * 128:(si + 1) * 128],
                            rhs=wo_sb[:, c2, half * 512:(half + 1) * 512],
                            start=(c2 == 0), stop=(c2 == 1),
                        )
                    # fold the output bias in here (nonzero only on core g==0)
                    nc.vector.tensor_tensor(
                        out=st["ob"][:, half * 512:(half + 1) * 512],
                        in0=pp,
                        in1=bo_bc[:, half * 512:(half + 1) * 512],
                        op=ALU.add,
                    )
                fadd(0.45, u_half)

            def u_dma(st=st, si=si):
                nc.scalar.dma_start(
                    out[si * 128:(si + 1) * 128, :], st["ob"][:]
                )
            fadd(0.0, u_dma)

    # ---- attention for one 512-query chunk ----
    def attention_chunk(qc, budget=0.6):
        n_k = 4 * qc + 4
        for p in range(2):
            ensure_g(("Q", qc, p))
            po = ps_po.tile([128, 2, 512], F32, tag="po")
            pend = []
            pt_cur = None
            for kt in range(n_k):
                ensure_g(("K", kt // 4, p))
                diag = kt >= 4 * qc
                q0 = 128 * (kt - 4 * qc) if diag else 0
                e = kt % 2
                if e == 0:
                    pt_cur = ptp.tile([128, 2, 2, 512], BF16, tag="pt")
                # scores for both heads of the pair: concurrent 64-row groups
                ps = ps_sc.tile([128, 2, 512], F32, tag="st")
                for h in range(2):
                    base = 64 * h
                    nc.tensor.matmul(
                        ps[:, h, q0:512],
                        lhsT=kT[p][base:base + 64, kt * 128:(kt + 1) * 128],
                        rhs=qT[p][base:base + 64, qc * 512 + q0:(qc + 1) * 512],
                        start=True, stop=True,
                    )
                # one exp for both heads
                nc.scalar.activation(
                    out=pt_cur[:, e, :, q0:512], in_=ps[:, :, q0:512],
                    func=AF.Exp, scale=SCALE,
                )
                if diag:
                    for h in range(2):
                        nc.vector.tensor_tensor(
                            out=pt_cur[:, e, h, q0:q0 + 128],
                            in0=pt_cur[:, e, h, q0:q0 + 128],
                            in1=mtri_sb[:],
                            op=ALU.mult,
                        )
                pend.append((kt, pt_cur, e, q0))
                # PV one step behind so exp(kt) overlaps scores(kt+1)
                if kt >= 1:
                    pkt, ptt, pe, pq0 = pend.pop(0)
                    ensure_g(("V", pkt // 4, pkt % 4))
                    for h in range(2):
                        hh = 2 * p + h
                        nc.tensor.matmul(
                            po[0:65, h, pq0:512],
                            lhsT=v_sb[:, pkt, HOFF[hh]:HOFF[hh] + 65],
                            rhs=ptt[:, pe, h, pq0:512],
                            start=(pkt == 0), stop=False,
                        )
                pop_fillers(budget)
            # final PV
            pkt, ptt, pe, pq0 = pend.pop(0)
            ensure_g(("V", pkt // 4, pkt % 4))
            for h in range(2):
                hh = 2 * p + h
                nc.tensor.matmul(
                    po[0:65, h, pq0:512],
                    lhsT=v_sb[:, pkt, HOFF[hh]:HOFF[hh] + 65],
                    rhs=ptt[:, pe, h, pq0:512],
                    start=(pkt == 0), stop=True,
                )
            # normalize: copy the 65 rows out (frees po), then O^T / den
            ocp = ocpp.tile([65, 2, 512], F32, tag="ocp")
            nc.vector.tensor_copy(out=ocp[:], in_=po[0:65, :, :])
            den0 = smallp.tile([1, 2, 512], F32, tag="den")
            nc.gpsimd.dma_start(den0[:], ocp[64:65, :, :])
            rec = smallp.tile([1, 2, 512], F32, tag="rec")
            nc.vector.reciprocal_approx_fast(out=rec[:], in_=den0[:])
            recbc = smallp.tile([64, 2, 512], F32, tag="recbc")
            nc.gpsimd.partition_broadcast(recbc[:], rec[:])
            # even head -> s1T rows 0..63 directly
            nc.vector.tensor_tensor(
                out=s1T[0:64, p, qc * 512:(qc + 1) * 512],
                in0=ocp[0:64, 0, :], in1=recbc[:, 0, :], op=ALU.mult,
            )
            # odd head: normalize at base 0, DMA to partitions 64..127
            tmp = smallp.tile([64, 512], BF16, tag="otmp")
            nc.vector.tensor_tensor(
                out=tmp[:], in0=ocp[0:64, 1, :], in1=recbc[:, 1, :], op=ALU.mult,
            )
            nc.gpsimd.dma_start(
                s1T[64:128, p, qc * 512:(qc + 1) * 512], tmp[:]
            )

    # ---- the pipeline: ascending chunks; all projections ride along as
    # gated fillers, pulled just ahead of the attention steps that consume
    # them; output projections of earlier chunks fill later windows ----
    for qc in range(QC):
        add_proj_chunk(qc)
    attention_chunk(0, budget=1.2)
    add_outproj_chunk(0)
    attention_chunk(1, budget=1.2)
    add_outproj_chunk(1)
    attention_chunk(2, budget=0.9)
    add_outproj_chunk(2)
    attention_chunk(3, budget=0.9)
    drain_fillers()
    add_outproj_chunk(3, use_sc=True)
    drain_fillers()

    ctx.close()


_NC_CACHE = None


def _get_nc():
    global _NC_CACHE
    if _NC_CACHE is None:
        _NC_CACHE = _build()
    return _NC_CACHE


def _make_in_maps(x_q, x_k, x_v, Wq, bq, Wk, bk, Wv, bv, Wo, bo):
    f32 = np.float32
    bf16 = ml_dtypes.bfloat16
    mtri_np = np.triu(np.ones((128, 128), f32)).astype(bf16)

    # per-batch transposed inputs (shared by the 4 cores of each batch)
    xT = {}
    for b in range(B):
        xT[("q", b)] = np.ascontiguousarray(np.asarray(x_q[b], f32).T).astype(bf16)
        xT[("k", b)] = np.ascontiguousarray(np.asarray(x_k[b], f32).T).astype(bf16)
        xT[("v", b)] = np.ascontiguousarray(np.asarray(x_v[b], f32).T).astype(bf16)

    def sb_layout(w):
        """[D_EMB, n] -> [128, ET*n]: partition p holds rows {t*128+p} packed
        contiguously, so the DMA is one max-length line per partition."""
        n = w.shape[1]
        return np.ascontiguousarray(
            w.reshape(ET, 128, n).transpose(1, 0, 2).reshape(128, ET * n)
        )

    in_maps = []
    for core in range(NCORES):
        b, g = core // 4, core % 4
        sl = slice(g * DM_L, (g + 1) * DM_L)
        # augmented V weight/bias
        wv_aug = np.zeros((D_EMB, WV_AUG), f32)
        bv_aug = np.zeros((WV_AUG,), f32)
        for h in range(HG):
            gh = g * HG + h
            o = HOFF[h]
            wv_aug[:, o:o + 64] = Wv[:, gh * DH:(gh + 1) * DH]
            bv_aug[o:o + 64] = bv[gh * DH:(gh + 1) * DH]
            bv_aug[o + 64] = 1.0
        wo_sl = np.asarray(Wo[sl, :], f32)  # [256, 1024]
        wo_c = np.ascontiguousarray(
            wo_sl.reshape(2, 128, D_OUT).transpose(1, 0, 2).reshape(128, 2 * D_OUT)
        )
        in_maps.append({
            "xqT": xT[("q", b)],
            "xkT": xT[("k", b)],
            "xvT": xT[("v", b)],
            "wq": sb_layout(np.asarray(Wq[:, sl], f32)).astype(bf16),
            "wk": sb_layout(np.asarray(Wk[:, sl], f32)).astype(bf16),
            "wv": sb_layout(wv_aug).astype(bf16),
            "bq": np.ascontiguousarray(bq[sl], f32),
            "bk": np.ascontiguousarray(bk[sl], f32),
            "bv": bv_aug,
            "wo": wo_c.astype(bf16),
            # bias folded into the partials by exactly one core per group
            "bo": np.ascontiguousarray(bo, f32) if g == 0
                  else np.zeros((D_OUT,), f32),
            "mtri": mtri_np,
        })
    return in_maps


def run(inputs, trace=False, trace_kwargs=None):
    """Run on 8 NeuronCores. Returns (output [2,2048,1024] f32, BassKernelResults)."""
    inputs = {k: np.asarray(v) for k, v in inputs.items()}
    nc = _get_nc()
    in_maps = _make_in_maps(
        inputs["x_q"], inputs["x_k"], inputs["x_v"],
        inputs["Wq"], inputs["bq"], inputs["Wk"], inputs["bk"],
        inputs["Wv"], inputs["bv"], inputs["Wo"], inputs["bo"],
    )
    kwargs = {}
    if trace:
        kwargs["trace"] = True
        if trace_kwargs:
            kwargs.update(trace_kwargs)
    res = run_bass_kernel_spmd(nc, in_maps, core_ids=list(range(NCORES)), **kwargs)
    # unshard: each core holds a full-shape row-parallel partial for its
    # batch (4 head-groups per batch); summing them is the unshard step.
    out_full = np.zeros((B, S, D_OUT), np.float32)
    for core in range(NCORES):
        b = core // 4
        out_full[b] += np.asarray(res.results[core]["out"], np.float32)
    return out_full, res


def kernel(**inputs) -> np.ndarray:
    out, _ = run(inputs, trace=False)
    return out


# revision 41
# speedup vs baseline: 1.0004x; 1.0004x over previous
"""MultiHeadAttention Trainium2 kernel, 8-way sharded (batch x head-group).

Sharding: core = 4*b + g  (b in {0,1} batch, g in {0..3} head-group of 4 heads).
Data parallel on batch; tensor parallel on heads for the Q/K/V projections
with a row-parallel Wo output projection.  Each core computes a full-shape
partial output for its batch (bias folded in on one core per group); the
host-side unshard step sums the 4 head-group partials per batch.

Device-side structure (all bf16 matmul operands, f32 PSUM accumulate):
  - x arrives pre-transposed and pre-cast on the host (xT [1024, 2048] bf16
    per batch): zero on-device transposes or input casts.  Weights arrive
    bf16 in SBUF-native layouts (one contiguous DMA line per partition).
  - Software pipeline over 512-row chunks, attention ascending 0..3; the
    K/V/Q projections of later chunks and the output projections of earlier
    chunks are interleaved into the ACT-bound attention kt-steps as small
    "filler" units, which keeps the PE dense (HAM stays un-throttled) and
    the scores->exp->PV chain saturated.  Emission-order gates (`ensure_kv`)
    guarantee a chunk's K/V projections are emitted before the kt steps
    that read them.
  - exp batched over both heads of a pair: scores for 2 heads land in one
    [128,2,512] PSUM tile (2 banks), one ACTIVATE computes both.
  - Softmax denominator folded into PV via an augmented ones column in V
    (row 64 of the PV accumulator); normalize copies the 65-row result out
    of PSUM in one DVE op (frees the bank fast), then reciprocal + gpsimd
    partition-broadcast + two DVE mults.
"""
import sys

for _p in ("/opt/trn_rl_repo",):
    if _p not in sys.path:
        sys.path.insert(0, _p)

from collections import deque

import numpy as np
import ml_dtypes

import concourse.bass as bass
import concourse.tile as tile
from concourse import bacc, mybir
from concourse.bass_utils import run_bass_kernel_spmd


def _install_ntff_hook_shim():
    """The agent container's antenv lacks axon_hooks; recreate it so
    run_bass_kernel_spmd(trace=True) can profile via the axon .so."""
    import types, contextlib, ctypes, os

    if "antenv.axon_hooks" in sys.modules:
        return
    mod = types.ModuleType("antenv.axon_hooks")
    _store = {"hook": None}
    mod.set_axon_ntff_profile_hook = lambda h: _store.__setitem__("hook", h)
    mod.get_axon_ntff_profile_hook = lambda: _store["hook"]
    sys.modules["antenv.axon_hooks"] = mod

    so_path = "/opt/axon/libaxon_pjrt.so"
    if not os.path.exists(so_path):
        return
    try:
        lib = ctypes.CDLL(so_path)
        if not hasattr(lib, "axon_start_nrt_profile"):
            return
        lib.axon_start_nrt_profile.argtypes = [
            ctypes.POINTER(ctypes.c_int64), ctypes.c_size_t]
        lib.axon_start_nrt_profile.restype = ctypes.c_int64
        lib.axon_stop_nrt_profile.argtypes = [ctypes.c_char_p]
        lib.axon_stop_nrt_profile.restype = ctypes.c_int64

        @contextlib.contextmanager
        def _hook(output_dir, device_ids):
            import jax
            jax.devices()
            if device_ids:
                ids = (ctypes.c_int64 * len(device_ids))(*device_ids)
                rc = lib.axon_start_nrt_profile(ids, len(device_ids))
            else:
                rc = lib.axon_start_nrt_profile(None, 0)
            if rc != 0:
                raise RuntimeError(f"axon_start_nrt_profile rc={rc}")
            try:
                yield
            finally:
                n = lib.axon_stop_nrt_profile(str(output_dir).encode())
                print(f"ntff profile: {n} file(s) written to {output_dir}")

        mod.set_axon_ntff_profile_hook(_hook)
    except Exception:
        pass


_install_ntff_hook_shim()

F32 = mybir.dt.float32
BF16 = mybir.dt.bfloat16
AF = mybir.ActivationFunctionType
ALU = mybir.AluOpType

B, S, D_EMB = 2, 2048, 1024
H, DH = 16, 64
HG = 4              # heads per core
DM_L = HG * DH      # 256 local mid dim
D_OUT = 1024
NCORES = 8
ET = D_EMB // 128   # 8 emb tiles
QC = 4              # q chunks of 512
SCALE = 1.0 / 8.0   # 1/sqrt(DH)

# augmented V layout: per head slice [v(64), one] -> PV output rows 0..63 = O,
# row 64 = softmax denominator (the ones column sums P over keys).
HOFF = [0, 65, 130, 195]
WV_AUG = 272        # 260 used + pad
WV_USED = 260


def _build():
    nc = bacc.Bacc(None, target_bir_lowering=False, num_devices=NCORES)

    xqT = nc.declare_dram_parameter("xqT", [D_EMB, S], BF16, isOutput=False)
    xkT = nc.declare_dram_parameter("xkT", [D_EMB, S], BF16, isOutput=False)
    xvT = nc.declare_dram_parameter("xvT", [D_EMB, S], BF16, isOutput=False)
    wq = nc.declare_dram_parameter("wq", [128, ET * DM_L], BF16, isOutput=False)
    wk = nc.declare_dram_parameter("wk", [128, ET * DM_L], BF16, isOutput=False)
    wv = nc.declare_dram_parameter("wv", [128, ET * WV_AUG], BF16, isOutput=False)
    bq = nc.declare_dram_parameter("bq", [DM_L], F32, isOutput=False)
    bk = nc.declare_dram_parameter("bk", [DM_L], F32, isOutput=False)
    bv = nc.declare_dram_parameter("bv", [WV_AUG], F32, isOutput=False)
    wo = nc.declare_dram_parameter("wo", [128, 2 * D_OUT], BF16, isOutput=False)
    bo = nc.declare_dram_parameter("bo", [D_OUT], F32, isOutput=False)
    mtri = nc.declare_dram_parameter("mtri", [128, 128], BF16, isOutput=False)
    out = nc.declare_dram_parameter("out", [S, D_OUT], BF16, isOutput=True)

    with tile.TileContext(nc) as tc:
        _emit(nc, tc, xqT.ap(), xkT.ap(), xvT.ap(), wq.ap(), wk.ap(), wv.ap(),
              bq.ap(), bk.ap(), bv.ap(), wo.ap(), bo.ap(), mtri.ap(), out.ap())
    nc.compile()
    return nc


def _emit(nc, tc, xqT, xkT, xvT, wq, wk, wv, bq, bk, bv, wo, bo, mtri, out):
    from contextlib import ExitStack

    ctx = ExitStack()
    consts = ctx.enter_context(tc.tile_pool(name="consts", bufs=1))
    wpool = ctx.enter_context(tc.tile_pool(name="wpool", bufs=1))
    persist = ctx.enter_context(tc.tile_pool(name="persist", bufs=1))
    xload = ctx.enter_context(tc.tile_pool(name="xload", bufs=6))
    ptp = ctx.enter_context(tc.tile_pool(name="ptp", bufs=3))
    ocpp = ctx.enter_context(tc.tile_pool(name="ocpp", bufs=2))
    smallp = ctx.enter_context(tc.tile_pool(name="smallp", bufs=4))
    outp = ctx.enter_context(tc.tile_pool(name="outp", bufs=4))
    ps_sc = ctx.enter_context(tc.tile_pool(name="ps_sc", bufs=2, space="PSUM"))
    ps_po = ctx.enter_context(tc.tile_pool(name="ps_po", bufs=1, space="PSUM"))
    ps_pp = ctx.enter_context(tc.tile_pool(name="ps_pp", bufs=2, space="PSUM"))

    # ---- first weight load goes out before everything else ----
    wk_sb = wpool.tile([128, ET, DM_L], BF16, name="wk")
    nc.scalar.dma_start(wk_sb[:], wk.rearrange("p (t d) -> p t d", t=ET))

    # ---- constants ----
    mtri_sb = consts.tile([128, 128], BF16)
    nc.scalar.dma_start(mtri_sb[:], mtri[:])

    # PE warm-up: back-to-back matmuls so HAM unthrottles while DMAs land.
    warm_ps = ps_pp.tile([128, 512], F32, tag="pp", name="warm")
    for _ in range(28):
        nc.tensor.matmul(
            warm_ps[:, 0:128], lhsT=mtri_sb[:], rhs=mtri_sb[:],
            start=True, stop=True,
        )

    # preload the exp table early (first ACTIVATE triggers the table DMA)
    dummy_f32 = consts.tile([1, 16], F32)
    nc.vector.memset(dummy_f32[:], 0.0)
    dummy_o = consts.tile([1, 16], F32)
    nc.scalar.activation(out=dummy_o[:], in_=dummy_f32[:], func=AF.Exp, scale=1.0)

    # biases: bq/bk as per-partition columns [128, 2] (c2-major) folded into
    # the qT/kT evacuations; bv broadcast to all partitions and folded into
    # the V evacuation.
    bk_sb = consts.tile([128, 2], F32, name="bk")
    nc.scalar.dma_start(bk_sb[:], bk.rearrange("(c p) -> p c", p=128))
    bq_sb = consts.tile([128, 2], F32, name="bq")
    nc.scalar.dma_start(bq_sb[:], bq.rearrange("(c p) -> p c", p=128))
    bv_row = consts.tile([1, WV_AUG], F32, name="bv_row")
    nc.scalar.dma_start(bv_row[:], bv[None, :])
    bv_bc = consts.tile([128, WV_AUG], F32, name="bv_bc")
    nc.gpsimd.partition_broadcast(bv_bc[:], bv_row[:])

    # bo broadcast to 128 partitions (f32); folded into the output-projection
    # partials (only core g==0 of each group gets a nonzero bo).
    bo_bc = consts.tile([128, D_OUT], F32)
    bo_bcast_ap = bass.AP(tensor=bo.tensor, offset=bo.offset, ap=[[0, 128], [1, D_OUT]])
    nc.gpsimd.dma_start(out=bo_bc[:], in_=bo_bcast_ap)

    # ---- weights (bf16, SBUF-native layout, scalar DMA queue) ----
    wv_sb = wpool.tile([128, ET, WV_AUG], BF16, name="wv")
    nc.scalar.dma_start(wv_sb[:], wv.rearrange("p (t d) -> p t d", t=ET))
    wq_sb = wpool.tile([128, ET, DM_L], BF16, name="wq")
    nc.scalar.dma_start(wq_sb[:], wq.rearrange("p (t d) -> p t d", t=ET))
    wo_sb = wpool.tile([128, 2, D_OUT], BF16, name="wo")
    nc.scalar.dma_start(wo_sb[:], wo.rearrange("p (t d) -> p t d", t=2))

    # ---- persistent attention operands ----
    qT = [persist.tile([128, S], BF16, name=f"qT{i}") for i in range(2)]
    kT = [persist.tile([128, S], BF16, name=f"kT{i}") for i in range(2)]
    v_sb = persist.tile([128, 4 * QC, WV_AUG], BF16)
    s1T = persist.tile([128, 2, S], BF16, name="s1T")

    # ---- xT chunk loads: one DMA per (input, chunk), ascending chunks ----
    xsrc = {"q": xqT, "k": xkT, "v": xvT}
    xtiles = {}

    def load_x(key, qc, parts=2):
        t = xload.tile([128, ET, 512], BF16, tag="xT", name=f"xT_{key}{qc}")
        src = xsrc[key].rearrange("(t p) s -> p t s", p=128)
        # split DMAs so the projection can start on the first piece
        step = ET // parts
        for j in range(0, ET, step):
            nc.sync.dma_start(
                t[:, j:j + step, :], src[:, j:j + step, qc * 512:(qc + 1) * 512]
            )
        xtiles[(key, qc)] = t

    for qc in range(QC):
        parts = 4 if qc == 0 else 2
        load_x("k", qc, parts)
        load_x("q", qc, parts)
        load_x("v", qc, 2)

    # ---- filler machinery: small PE units interleaved into attention.
    # Emission order IS dependency order for the Tile framework, so a unit
    # that writes data consumed by an attention step must be emitted before
    # that step: `ensure_kv` force-pops K/V projection units up to the chunk
    # a kt step is about to read. ----
    fillers = deque()
    pending = {}

    def fadd(cost, fn, gate=None):
        fillers.append((cost, fn, gate))
        if gate is not None:
            pending[gate] = pending.get(gate, 0) + 1

    def pop_one():
        cost, fn, gate = fillers.popleft()
        fn()
        if gate is not None:
            pending[gate] -= 1
        return cost

    def pop_fillers(budget):
        while fillers and budget > 0.0:
            budget -= pop_one()

    def ensure_g(key):
        while fillers and pending.get(key, 0):
            pop_one()

    def drain_fillers():
        while fillers:
            pop_one()

    def add_projT_units(qc, c2, w_sb, b_sb, dst):
        """qT/kT projection for mid-half c2 of chunk qc -> dst[c2] columns."""
        st = {}
        kind = "Q" if dst is qT else "K"
        gate = (kind, qc, c2)
        xt = xtiles[(kind.lower(), qc)]

        def u_start(st=st, xt=xt, c2=c2, w_sb=w_sb):
            pp = ps_pp.tile([128, 512], F32, tag="pp")
            st["pp"] = pp
            nc.tensor.matmul(
                pp[:, 0:512],
                lhsT=w_sb[:, 0, c2 * 128:(c2 + 1) * 128],
                rhs=xt[:, 0, :],
                start=True, stop=False,
            )
        fadd(0.25, u_start, gate)

        for e0 in (1, 3, 5):
            def u_mid(st=st, xt=xt, c2=c2, w_sb=w_sb, e0=e0):
                for ei in (e0, e0 + 1):
                    nc.tensor.matmul(
                        st["pp"][:, 0:512],
                        lhsT=w_sb[:, ei, c2 * 128:(c2 + 1) * 128],
                        rhs=xt[:, ei, :],
                        start=False, stop=False,
                    )
            fadd(0.45, u_mid, gate)

        def u_end(st=st, xt=xt, c2=c2, w_sb=w_sb, dst=dst, qc=qc, b_sb=b_sb):
            nc.tensor.matmul(
                st["pp"][:, 0:512],
                lhsT=w_sb[:, ET - 1, c2 * 128:(c2 + 1) * 128],
                rhs=xt[:, ET - 1, :],
                start=False, stop=True,
            )
            nc.vector.tensor_scalar(
                out=dst[c2][:, qc * 512:(qc + 1) * 512],
                in0=st["pp"][:, 0:512],
                scalar1=b_sb[:, c2:c2 + 1],
                scalar2=None,
                op0=ALU.add,
            )
        fadd(0.45, u_end, gate)

    def add_projV_units(qc, r):
        """V projection for 128-row block r of chunk qc -> v_sb natural."""
        st = {}
        si = 4 * qc + r
        xt = xtiles[("v", qc)]

        def u_start(st=st, xt=xt, r=r):
            pp = ps_pp.tile([128, 512], F32, tag="pp")
            st["pp"] = pp
            nc.tensor.matmul(
                pp[:, 0:WV_USED],
                lhsT=xt[:, 0, r * 128:(r + 1) * 128],
                rhs=wv_sb[:, 0, 0:WV_USED],
                start=True, stop=False,
            )
        fadd(0.15, u_start, ("V", qc, r))

        for e0 in (1, 3, 5):
            def u_mid(st=st, xt=xt, r=r, e0=e0):
                for ei in (e0, e0 + 1):
                    nc.tensor.matmul(
                        st["pp"][:, 0:WV_USED],
                        lhsT=xt[:, ei, r * 128:(r + 1) * 128],
                        rhs=wv_sb[:, ei, 0:WV_USED],
                        start=False, stop=False,
                    )
            fadd(0.3, u_mid, ("V", qc, r))

        def u_end(st=st, xt=xt, r=r, si=si):
            nc.tensor.matmul(
                st["pp"][:, 0:WV_USED],
                lhsT=xt[:, ET - 1, r * 128:(r + 1) * 128],
                rhs=wv_sb[:, ET - 1, 0:WV_USED],
                start=False, stop=True,
            )
            nc.vector.tensor_tensor(
                out=v_sb[:, si, 0:WV_USED],
                in0=st["pp"][:, 0:WV_USED],
                in1=bv_bc[:, 0:WV_USED],
                op=ALU.add,
            )
        fadd(0.3, u_end, ("V", qc, r))

    def add_proj_chunk(qc):
        add_projT_units(qc, 0, wk_sb, bk_sb, kT)
        add_projT_units(qc, 0, wq_sb, bq_sb, qT)
        for r in range(4):
            add_projV_units(qc, r)
        add_projT_units(qc, 1, wk_sb, bk_sb, kT)
        add_projT_units(qc, 1, wq_sb, bq_sb, qT)

    def add_outproj_chunk(qc, use_sc=False):
        """Output projection for 512-row block qc; partial rows go straight
        to the output parameter (host sums the 4 head-group partials)."""
        for r in range(4):
            si = 4 * qc + r
            st = {}

            def u_alloc(st=st):
                st["ob"] = outp.tile([128, D_OUT], BF16, tag="ob", name="ob")
            fadd(0.0, u_alloc)

            for half in range(2):
                def u_half(st=st, si=si, half=half, r=r):
                    if use_sc and (r + half) % 2 == 0:
                        ppt = ps_sc.tile([128, 2, 512], F32, tag="st", name="pp2")
                        pp = ppt[:, 0, 0:512]
                    else:
                        ppt = ps_pp.tile([128, 512], F32, tag="pp")
                        pp = ppt[:, 0:512]
                    for c2 in range(2):
                        nc.tensor.matmul(
                            pp,
                            lhsT=s1T[:, c2, si * 128:(si + 1) * 128],
                            rhs=wo_sb[:, c2, half * 512:(half + 1) * 512],
                            start=(c2 == 0), stop=(c2 == 1),
                        )
                    # fold the output bias in here (nonzero only on core g==0)
                    nc.vector.tensor_tensor(
                        out=st["ob"][:, half * 512:(half + 1) * 512],
                        in0=pp,
                        in1=bo_bc[:, half * 512:(half + 1) * 512],
                        op=ALU.add,
                    )
                fadd(0.45, u_half)

            def u_dma(st=st, si=si):
                nc.scalar.dma_start(
                    out[si * 128:(si + 1) * 128, :], st["ob"][:]
                )
            fadd(0.0, u_dma)

    # ---- attention for one 512-query chunk ----
    def attention_chunk(qc, budget=0.6):
        n_k = 4 * qc + 4
        for p in range(2):
            ensure_g(("Q", qc, p))
            po = ps_po.tile([128, 2, 512], F32, tag="po")
            pend = []
            pt_cur = None
            for kt in range(n_k):
                ensure_g(("K", kt // 4, p))
                diag = kt >= 4 * qc
                q0 = 128 * (kt - 4 * qc) if diag else 0
                e = kt % 2
                if e == 0:
                    pt_cur = ptp.tile([128, 2, 2, 512], BF16, tag="pt")
                # scores for both heads of the pair: concurrent 64-row groups
                ps = ps_sc.tile([128, 2, 512], F32, tag="st")
                for h in range(2):
                    base = 64 * h
                    nc.tensor.matmul(
                        ps[:, h, q0:512],
                        lhsT=kT[p][base:base + 64, kt * 128:(kt + 1) * 128],
                        rhs=qT[p][base:base + 64, qc * 512 + q0:(qc + 1) * 512],
                        start=True, stop=True,
                    )
                # one exp for both heads
                nc.scalar.activation(
                    out=pt_cur[:, e, :, q0:512], in_=ps[:, :, q0:512],
                    func=AF.Exp, scale=SCALE,
                )
                if diag:
                    for h in range(2):
                        nc.vector.tensor_tensor(
                            out=pt_cur[:, e, h, q0:q0 + 128],
                            in0=pt_cur[:, e, h, q0:q0 + 128],
                            in1=mtri_sb[:],
                            op=ALU.mult,
                        )
                pend.append((kt, pt_cur, e, q0))
                # PV one step behind so exp(kt) overlaps scores(kt+1)
                if kt >= 1:
                    pkt, ptt, pe, pq0 = pend.pop(0)
                    ensure_g(("V", pkt // 4, pkt % 4))
                    for h in range(2):
                        hh = 2 * p + h
                        nc.tensor.matmul(
                            po[0:65, h, pq0:512],
                            lhsT=v_sb[:, pkt, HOFF[hh]:HOFF[hh] + 65],
                            rhs=ptt[:, pe, h, pq0:512],
                            start=(pkt == 0), stop=False,
                        )
                pop_fillers(budget)
            # final PV
            pkt, ptt, pe, pq0 = pend.pop(0)
            ensure_g(("V", pkt // 4, pkt % 4))
            for h in range(2):
                hh = 2 * p + h
                nc.tensor.matmul(
                    po[0:65, h, pq0:512],
                    lhsT=v_sb[:, pkt, HOFF[hh]:HOFF[hh] + 65],
                    rhs=ptt[:, pe, h, pq0:512],
                    start=(pkt == 0), stop=True,
                )
            # normalize: copy the 65 rows out (frees po), then O^T / den
            ocp = ocpp.tile([65, 2, 512], F32, tag="ocp")
            nc.vector.tensor_copy(out=ocp[:], in_=po[0:65, :, :])
            den0 = smallp.tile([1, 2, 512], F32, tag="den")
            if qc == QC - 1 and p == 1:
                # tail: ACT is idle; skip the SBUF-SBUF DMA hop
                nc.scalar.copy(out=den0[:], in_=po[64:65, :, :])
            else:
                nc.gpsimd.dma_start(den0[:], ocp[64:65, :, :])
            rec = smallp.tile([1, 2, 512], F32, tag="rec")
            nc.vector.reciprocal_approx_fast(out=rec[:], in_=den0[:])
            recbc = smallp.tile([64, 2, 512], F32, tag="recbc")
            nc.gpsimd.partition_broadcast(recbc[:], rec[:])
            # even head -> s1T rows 0..63 directly
            nc.vector.tensor_tensor(
                out=s1T[0:64, p, qc * 512:(qc + 1) * 512],
                in0=ocp[0:64, 0, :], in1=recbc[:, 0, :], op=ALU.mult,
            )
            # odd head: normalize at base 0, DMA to partitions 64..127
            tmp = smallp.tile([64, 512], BF16, tag="otmp")
            nc.vector.tensor_tensor(
                out=tmp[:], in0=ocp[0:64, 1, :], in1=recbc[:, 1, :], op=ALU.mult,
            )
            nc.gpsimd.dma_start(
                s1T[64:128, p, qc * 512:(qc + 1) * 512], tmp[:]
            )

    # ---- the pipeline: ascending chunks; all projections ride along as
    # gated fillers, pulled just ahead of the attention steps that consume
    # them; output projections of earlier chunks fill later windows ----
    for qc in range(QC):
        add_proj_chunk(qc)
    attention_chunk(0, budget=0.7)
    add_outproj_chunk(0)
    attention_chunk(1, budget=0.9)
    add_outproj_chunk(1)
    attention_chunk(2, budget=0.9)
    add_outproj_chunk(2)
    attention_chunk(3, budget=0.9)
    drain_fillers()
    add_outproj_chunk(3, use_sc=True)
    drain_fillers()

    ctx.close()


_NC_CACHE = None


def _get_nc():
    global _NC_CACHE
    if _NC_CACHE is None:
        _NC_CACHE = _build()
    return _NC_CACHE


def _make_in_maps(x_q, x_k, x_v, Wq, bq, Wk, bk, Wv, bv, Wo, bo):
    f32 = np.float32
    bf16 = ml_dtypes.bfloat16
    mtri_np = np.triu(np.ones((128, 128), f32)).astype(bf16)

    # per-batch transposed inputs (shared by the 4 cores of each batch)
    xT = {}
    for b in range(B):
        xT[("q", b)] = np.ascontiguousarray(np.asarray(x_q[b], f32).T).astype(bf16)
        xT[("k", b)] = np.ascontiguousarray(np.asarray(x_k[b], f32).T).astype(bf16)
        xT[("v", b)] = np.ascontiguousarray(np.asarray(x_v[b], f32).T).astype(bf16)

    def sb_layout(w):
        """[D_EMB, n] -> [128, ET*n]: partition p holds rows {t*128+p} packed
        contiguously, so the DMA is one max-length line per partition."""
        n = w.shape[1]
        return np.ascontiguousarray(
            w.reshape(ET, 128, n).transpose(1, 0, 2).reshape(128, ET * n)
        )

    in_maps = []
    for core in range(NCORES):
        b, g = core // 4, core % 4
        sl = slice(g * DM_L, (g + 1) * DM_L)
        # augmented V weight/bias
        wv_aug = np.zeros((D_EMB, WV_AUG), f32)
        bv_aug = np.zeros((WV_AUG,), f32)
        for h in range(HG):
            gh = g * HG + h
            o = HOFF[h]
            wv_aug[:, o:o + 64] = Wv[:, gh * DH:(gh + 1) * DH]
            bv_aug[o:o + 64] = bv[gh * DH:(gh + 1) * DH]
            bv_aug[o + 64] = 1.0
        wo_sl = np.asarray(Wo[sl, :], f32)  # [256, 1024]
        wo_c = np.ascontiguousarray(
            wo_sl.reshape(2, 128, D_OUT).transpose(1, 0, 2).reshape(128, 2 * D_OUT)
        )
        in_maps.append({
            "xqT": xT[("q", b)],
            "xkT": xT[("k", b)],
            "xvT": xT[("v", b)],
            "wq": sb_layout(np.asarray(Wq[:, sl], f32)).astype(bf16),
            "wk": sb_layout(np.asarray(Wk[:, sl], f32)).astype(bf16),
            "wv": sb_layout(wv_aug).astype(bf16),
            "bq": np.ascontiguousarray(bq[sl], f32),
            "bk": np.ascontiguousarray(bk[sl], f32),
            "bv": bv_aug,
            "wo": wo_c.astype(bf16),
            # bias folded into the partials by exactly one core per group
            "bo": np.ascontiguousarray(bo, f32) if g == 0
                  else np.zeros((D_OUT,), f32),
            "mtri": mtri_np,
        })
    return in_maps


def run(inputs, trace=False, trace_kwargs=None):
    """Run on 8 NeuronCores. Returns (output [2,2048,1024] f32, BassKernelResults)."""
    inputs = {k: np.asarray(v) for k, v in inputs.items()}
    nc = _get_nc()
    in_maps = _make_in_maps(
        inputs["x_q"], inputs["x_k"], inputs["x_v"],
        inputs["Wq"], inputs["bq"], inputs["Wk"], inputs["bk"],
        inputs["Wv"], inputs["bv"], inputs["Wo"], inputs["bo"],
    )
    kwargs = {}
    if trace:
        kwargs["trace"] = True
        if trace_kwargs:
            kwargs.update(trace_kwargs)
    res = run_bass_kernel_spmd(nc, in_maps, core_ids=list(range(NCORES)), **kwargs)
    # unshard: each core holds a full-shape row-parallel partial for its
    # batch (4 head-groups per batch); summing them is the unshard step.
    out_full = np.zeros((B, S, D_OUT), np.float32)
    for core in range(NCORES):
        b = core // 4
        out_full[b] += np.asarray(res.results[core]["out"], np.float32)
    return out_full, res


def kernel(**inputs) -> np.ndarray:
    out, _ = run(inputs, trace=False)
    return out
